# revision 54
# baseline (speedup 1.0000x reference)
"""Low-rank cross-attention on 8 Trainium2 NeuronCores (Bass/Tile).

Problem: out = (softmax((tgt@Wq.T)(memory@Wk.T).T / sqrt(r)) @ (memory@Wv.T)) @ Wo.T
Shapes: tgt/memory [4, 2048, 1024], r=128, d_model=1024.

Sharding: core c in 0..7 handles batch b=c//2 and query-half h=c%2
(1024 query tokens) against the full 2048-token memory of its batch.
No collectives.

Key algebraic move: reassociate the value/output path
    out = attn @ (mem @ Wv.T) @ Wo.T = (attn @ mem) @ (Wo @ Wv).T
so the 2.1-GMAC value projection disappears. W2 = Wo@Wv is weight-only,
so it is folded on the host (standard inference weight folding, like the
transposes/casts already done there); the device contracts
Z = exp(scores) @ mem directly and projects once with W2.

Layouts (contraction dim always on the SBUF partition axis):
  qT [r,T]    = WqT.T @ tgtT          (contract d)
  kT [r,S]    = WkT.T @ memT          (contract d)
  expT[S,Tq]  = exp(scale * kT_s.T @ qT)   (contract r, single MM per s-tile)
  ZT [d,Tq]   = mem_s.T @ expT        (contract s) -- mem in natural [S,d]
  out [T,o]   = ZT_t.T @ W2T          (contract d)
Softmax: logits are bounded (|x| < ~10) so exp is fp32-safe with no max
subtraction; row-sums come from a ones-vector matmul and the division is
folded into the final PSUM->SBUF scaling (per-partition scalar multiply).

PSUM discipline: `start=True` clears has_written for the WHOLE bank, so
accumulation groups sharing a bank must run sequentially, never
interleaved.  Budget (8 banks): scores 2, zt/sums 3 (tag round-robin,
sums first), out 3 -- every phase multi-buffers so the next group's MMs
overlap the previous group's PSUM drain.

DMA: inputs load as ~12 large coalesced transfers (3D stacked-tile APs)
in consumption order -- the SP-queue issue cost (~0.6us/instruction)
makes many small loads issue-bound.  Per-512-column destination tiles
keep the dependency grain fine so projections start while later inputs
are still in flight.
"""

import ml_dtypes
import numpy as np

import concourse.bacc as bacc
import concourse.bass as bass
import concourse.mybir as mybir
import concourse.tile as tile
from concourse.bass_utils import run_bass_kernel_spmd

FP = mybir.dt.float32
BF = mybir.dt.bfloat16
ts = bass.ts

B = 4
T_FULL = 2048
D = 1024
R = 128
S = 2048
O = 1024
T = 1024            # per-core query tokens (half of T_FULL)
P = 128
SCALE = 1.0 / np.sqrt(128.0)

KD = D // P         # 8 contraction tiles over d
NS = S // P         # 16 key tiles
ND = D // P         # 8 d tiles (Z features)
TQ = 512            # query-column strip processed per attention pass
NQ = T // TQ        # 2 strips

# Set by test harness to enable NTFF profiling; LAST_RESULT holds the
# BassKernelResults of the most recent kernel() call.
TRACE = False
LAST_RESULT = None
_PROG = None


def _build_program():
    nc = bacc.Bacc()

    tgtT_d = nc.dram_tensor("tgtT", [D, T], BF, kind="ExternalInput")
    memT_d = nc.dram_tensor("memT", [D, S], BF, kind="ExternalInput")
    mem_d = nc.dram_tensor("mem", [S, D], BF, kind="ExternalInput")
    wq_d = nc.dram_tensor("WqT", [D, R], BF, kind="ExternalInput")
    wk_d = nc.dram_tensor("WkT", [D, R], BF, kind="ExternalInput")
    w2_d = nc.dram_tensor("W2T", [D, O], BF, kind="ExternalInput")
    out_d = nc.dram_tensor("out", [T, O], BF, kind="ExternalOutput")

    Exp = mybir.ActivationFunctionType.Exp

    with tile.TileContext(nc) as tc:
        with tc.tile_pool(name="perm", bufs=1) as perm, \
             tc.tile_pool(name="dram", bufs=1, space="DRAM") as dpool, \
             tc.tile_pool(name="expp", bufs=1) as expp, \
             tc.tile_pool(name="ztsb", bufs=2) as ztsb, \
             tc.tile_pool(name="outp", bufs=3) as outp, \
             tc.tile_pool(name="rcp", bufs=8) as rcp:
            # qT/kT in per-512-col chunk tiles for fine dependency grain
            qT = [perm.tile([P, 512], BF, tag=f"qT{n}", name=f"qT{n}")
                  for n in range(T // 512)]
            kT = [perm.tile([P, 512], BF, tag=f"kT{n}", name=f"kT{n}")
                  for n in range(S // 512)]
            ones = perm.tile([P, 1], BF, tag="ones")
            nc.vector.memset(ones, 1.0)
            sums_d = dpool.tile([1, T], FP)

            # Coalesced input loads: the SP-queue DMA issue costs ~0.6us
            # per instruction, so small per-tile loads are issue-bound.
            # Each load below is one big DMA whose (k p) c -> p (k c)
            # rearrange stacks the 128-row DRAM tiles along the free dim.
            # Per-chunk destination tiles keep the dependency grain fine.
            memT = [perm.tile([P, KD * 512], BF, tag=f"m{n}", name=f"m{n}")
                    for n in range(S // 512)]
            memb = [perm.tile([P, 8 * D], BF, tag=f"n{h}", name=f"n{h}")
                    for h in range(2)]
            tgt = [perm.tile([P, KD * 512], BF, tag=f"t{n}", name=f"t{n}")
                   for n in range(T // 512)]
            wq = perm.tile([P, KD * R], BF, tag="wq")
            wk = perm.tile([P, KD * R], BF, tag="wk")
            w2 = [perm.tile([P, 4 * O], BF, tag=f"w2{h}", name=f"w2{h}")
                  for h in range(2)]

            # DMA order = consumption order
            def stk(dst, src_ap, blk):
                # one DMA: stack 128-row DRAM tiles along the SBUF free dim
                nc.sync.dma_start(
                    out=dst.rearrange("p (k c) -> p k c", c=blk),
                    in_=src_ap.rearrange("(k p) c -> p k c", p=P))

            stk(wq, wq_d, R)
            stk(wk, wk_d, R)
            stk(tgt[0], tgtT_d[:, 0:512], 512)
            stk(memT[0], memT_d[:, 0:512], 512)
            stk(tgt[1], tgtT_d[:, 512:1024], 512)
            for n in range(1, S // 512):
                stk(memT[n], memT_d[:, ts(n, 512)], 512)
            for h in range(2):
                stk(memb[h], mem_d[ts(h, 1024), :], D)
            for h in range(2):
                stk(w2[h], w2_d[ts(h, 512), :], O)

            def mem_sl(s, dt):      # lhsT [s-tile, d-tile] slice of mem
                return memb[s // 8][:, (s % 8) * D + dt * P:
                                    (s % 8) * D + (dt + 1) * P]

            def w2_sl(dt, oh):      # rhs [d-tile, o-chunk] slice of W2T
                return w2[dt // 4][:, (dt % 4) * O + oh * 512:
                                   (dt % 4) * O + (oh + 1) * 512]

            # ---- projections qT, kT ----
            with tc.tile_pool(name="psA", bufs=4, space="PSUM") as psA:
                # PE warmup while the first DMAs land: HAM starts at
                # K=4/8 (1.2 GHz) and needs ~3.4us of activity to reach
                # full clock; these dummy MMs (deps: only the ones memset)
                # span the initial DMA wait so real MMs start warm.
                wup_sb = perm.tile([P, 128], BF, tag="wup_sb")
                nc.vector.memset(wup_sb, 1.0)
                wup = psA.tile([P, 128], FP, tag="wup")
                for _ in range(36):
                    nc.tensor.matmul(wup[0:1, :], ones, wup_sb,
                                     start=True, stop=True)
                for n in range(T // 512):
                    ps = psA.tile([P, 512], FP)
                    for k in range(KD):
                        nc.tensor.matmul(ps, wq[:, ts(k, R)],
                                         tgt[n][:, ts(k, 512)],
                                         start=(k == 0), stop=(k == KD - 1))
                    nc.vector.tensor_copy(qT[n], ps)

                for n in range(S // 512):
                    ps = psA.tile([P, 512], FP)
                    for k in range(KD):
                        nc.tensor.matmul(ps, wk[:, ts(k, R)],
                                         memT[n][:, ts(k, 512)],
                                         start=(k == 0), stop=(k == KD - 1))
                    nc.vector.tensor_copy(kT[n], ps)

            # ---- attention + output projection, per 512-col strip ----
            with tc.tile_pool(name="psc", bufs=2, space="PSUM") as psc, \
                 tc.tile_pool(name="pszt", bufs=1, space="PSUM") as pszt, \
                 tc.tile_pool(name="pso", bufs=3, space="PSUM") as pso:
                zt_tags = ["zta", "ztb", "ztc"]
                ztc = 2  # round-robin counter over zt banks

                # Phase 1 (both strips): scores + exp + row-sums.  These
                # depend only on qT/kT (early DMAs), giving the PE a long
                # runway of ready work while mem/W2 are still in flight.
                ex = [[None] * NS for _ in range(NQ)]
                for q in range(NQ):
                    sums_ps = pszt.tile([P, TQ], FP, tag=zt_tags[q])
                    for s in range(NS):
                        sc = psc.tile([P, TQ], FP)
                        nc.tensor.matmul(sc, kT[s // 4][:, ts(s % 4, P)],
                                         qT[q], start=True, stop=True)
                        e_t = expp.tile([P, TQ], BF, tag=f"x{q}_{s}",
                                        name=f"x{q}_{s}")
                        nc.scalar.activation(e_t, sc, Exp, scale=float(SCALE))
                        ex[q][s] = e_t
                        nc.tensor.matmul(sums_ps[0:1, :], ones, e_t,
                                         start=(s == 0), stop=(s == NS - 1))
                    ssb = rcp.tile([1, TQ], FP, tag="ssb")
                    nc.vector.tensor_copy(ssb, sums_ps[0:1, :])
                    nc.sync.dma_start(out=sums_d[0:1, ts(q, TQ)], in_=ssb)

                # Phase 2 (per strip): ZT accumulation, then the output
                # projection; strip q's out MMs overlap strip q+1's Z.
                for q in range(NQ):
                    zt_sb = [ztsb.tile([P, TQ], BF, tag=f"zs{d}",
                                       name=f"zs{d}") for d in range(ND)]
                    for dt in range(ND):
                        zt_ps = pszt.tile([P, TQ], FP, tag=zt_tags[ztc % 3])
                        ztc += 1
                        for s in range(NS):
                            nc.tensor.matmul(zt_ps, mem_sl(s, dt),
                                             ex[q][s], start=(s == 0),
                                             stop=(s == NS - 1))
                        nc.vector.tensor_copy(zt_sb[dt], zt_ps)

                    # per-query reciprocal of the row sums, transposed into
                    # partition layout via a DRAM bounce
                    rcs = []
                    for tt in range(TQ // P):
                        tg = q * (TQ // P) + tt
                        sload = rcp.tile([P, 1], FP, tag="sl")
                        nc.sync.dma_start(
                            out=sload,
                            in_=sums_d[0:1, ts(tg, P)].rearrange("a b -> b a"))
                        rc = rcp.tile([P, 1], FP, tag="rc")
                        nc.vector.reciprocal(rc, sload)
                        rcs.append(rc)

                    # out[t,o] = sum_d ZT[d,t-tile].T @ W2T[d,o-chunk]
                    for tt in range(TQ // P):
                        tg = q * (TQ // P) + tt
                        for oh in range(O // 512):
                            po = pso.tile([P, 512], FP)
                            for dt in range(ND):
                                nc.tensor.matmul(po, zt_sb[dt][:, ts(tt, P)],
                                                 w2_sl(dt, oh),
                                                 start=(dt == 0),
                                                 stop=(dt == ND - 1))
                            ob = outp.tile([P, 512], BF)
                            # alternate the scale-copy between DVE and ACT
                            # (ACT is idle by the out phase) so the final
                            # drain chain isn't serialized on one engine
                            if (tt + oh) % 2 == 0:
                                nc.vector.tensor_scalar_mul(ob, po, rcs[tt])
                            else:
                                nc.scalar.activation(
                                    ob, po, mybir.ActivationFunctionType.Copy,
                                    scale=rcs[tt])
                            # alternate store queues so the kernel-tail
                            # DMA drain halves (exp stream is done by now)
                            eng = nc.sync if (tt + oh) % 2 == 0 else nc.scalar
                            eng.dma_start(out=out_d[ts(tg, P), ts(oh, 512)],
                                          in_=ob)
    return nc


def kernel(tgt, memory, Wq, Wk, Wv, Wo):
    """8-way data-parallel (batch x query-half) low-rank cross-attention
    on the 8 NeuronCores via the hand-written Bass/Tile program above."""
    global LAST_RESULT, _PROG

    tgt = np.asarray(tgt, dtype=np.float32)
    memory = np.asarray(memory, dtype=np.float32)
    bf = ml_dtypes.bfloat16

    WqT = np.ascontiguousarray(np.asarray(Wq, np.float32).T).astype(bf)
    WkT = np.ascontiguousarray(np.asarray(Wk, np.float32).T).astype(bf)
    # weight folding: W2 = Wo @ Wv, device consumes W2T = Wv.T @ Wo.T
    W2T = np.ascontiguousarray(
        np.asarray(Wv, np.float32).T @ np.asarray(Wo, np.float32).T
    ).astype(bf)

    # core c -> batch c//2, query-half c%2
    in_maps = []
    for c in range(8):
        b, h = divmod(c, 2)
        mem_b = memory[b]
        in_maps.append({
            "tgtT": np.ascontiguousarray(tgt[b, h * T:(h + 1) * T, :].T)
                      .astype(bf),                      # [D, T]
            "memT": np.ascontiguousarray(mem_b.T).astype(bf),  # [D, S]
            "mem": np.ascontiguousarray(mem_b).astype(bf),     # [S, D]
            "WqT": WqT, "WkT": WkT, "W2T": W2T,
        })

    if _PROG is None:
        _PROG = _build_program()
        _PROG.finalize()
    res = run_bass_kernel_spmd(_PROG, in_maps, core_ids=list(range(8)),
                               trace=TRACE)
    LAST_RESULT = res

    out = np.empty((B, T_FULL, O), dtype=np.float32)
    for c in range(8):
        b, h = divmod(c, 2)
        out[b, h * T:(h + 1) * T, :] = res.results[c]["out"].astype(np.float32)
    return out


# revision 55
# speedup vs baseline: 1.0202x; 1.0202x over previous
"""Low-rank cross-attention on 8 Trainium2 NeuronCores (Bass/Tile).

Problem: out = (softmax((tgt@Wq.T)(memory@Wk.T).T / sqrt(r)) @ (memory@Wv.T)) @ Wo.T
Shapes: tgt/memory [4, 2048, 1024], r=128, d_model=1024.

Sharding: core c in 0..7 handles batch b=c//2 and query-half h=c%2
(1024 query tokens) against the full 2048-token memory of its batch.
No collectives.

Key algebraic move: reassociate the value/output path
    out = attn @ (mem @ Wv.T) @ Wo.T = (attn @ mem) @ (Wo @ Wv).T
so the 2.1-GMAC value projection disappears. W2 = Wo@Wv is weight-only,
so it is folded on the host (standard inference weight folding, like the
transposes/casts already done there); the device contracts
Z = exp(scores) @ mem directly and projects once with W2.

Layouts (contraction dim always on the SBUF partition axis):
  qT [r,T]    = WqT.T @ tgtT          (contract d)
  kT [r,S]    = WkT.T @ memT          (contract d)
  expT[S,Tq]  = exp(scale * kT_s.T @ qT)   (contract r, single MM per s-tile)
  ZT [d,Tq]   = mem_s.T @ expT        (contract s) -- mem in natural [S,d]
  out [T,o]   = ZT_t.T @ W2T          (contract d)
Softmax: logits are bounded (|x| < ~10) so exp is fp32-safe with no max
subtraction; row-sums come from a ones-vector matmul and the division is
folded into the final PSUM->SBUF scaling (per-partition scalar multiply).

PSUM discipline: `start=True` clears has_written for the WHOLE bank, so
accumulation groups sharing a bank must run sequentially, never
interleaved.  Budget (8 banks): scores 2, zt/sums 3 (tag round-robin,
sums first), out 3 -- every phase multi-buffers so the next group's MMs
overlap the previous group's PSUM drain.

DMA: inputs load as ~12 large coalesced transfers (3D stacked-tile APs)
in consumption order -- the SP-queue issue cost (~0.6us/instruction)
makes many small loads issue-bound.  Per-512-column destination tiles
keep the dependency grain fine so projections start while later inputs
are still in flight.
"""

import ml_dtypes
import numpy as np

import concourse.bacc as bacc
import concourse.bass as bass
import concourse.mybir as mybir
import concourse.tile as tile
from concourse.bass_utils import run_bass_kernel_spmd

FP = mybir.dt.float32
BF = mybir.dt.bfloat16
ts = bass.ts

B = 4
T_FULL = 2048
D = 1024
R = 128
S = 2048
O = 1024
T = 1024            # per-core query tokens (half of T_FULL)
P = 128
SCALE = 1.0 / np.sqrt(128.0)

KD = D // P         # 8 contraction tiles over d
NS = S // P         # 16 key tiles
ND = D // P         # 8 d tiles (Z features)
TQ = 512            # query-column strip processed per attention pass
NQ = T // TQ        # 2 strips

# Set by test harness to enable NTFF profiling; LAST_RESULT holds the
# BassKernelResults of the most recent kernel() call.
TRACE = False
LAST_RESULT = None
_PROG = None


def _build_program():
    nc = bacc.Bacc()

    tgtT_d = nc.dram_tensor("tgtT", [D, T], BF, kind="ExternalInput")
    memT_d = nc.dram_tensor("memT", [D, S], BF, kind="ExternalInput")
    mem_d = nc.dram_tensor("mem", [S, D], BF, kind="ExternalInput")
    wq_d = nc.dram_tensor("WqT", [D, R], BF, kind="ExternalInput")
    wk_d = nc.dram_tensor("WkT", [D, R], BF, kind="ExternalInput")
    w2_d = nc.dram_tensor("W2T", [D, O], BF, kind="ExternalInput")
    out_d = nc.dram_tensor("out", [T, O], BF, kind="ExternalOutput")

    Exp = mybir.ActivationFunctionType.Exp

    with tile.TileContext(nc) as tc:
        with tc.tile_pool(name="perm", bufs=1) as perm, \
             tc.tile_pool(name="dram", bufs=1, space="DRAM") as dpool, \
             tc.tile_pool(name="expp", bufs=1) as expp, \
             tc.tile_pool(name="ztsb", bufs=2) as ztsb, \
             tc.tile_pool(name="outp", bufs=3) as outp, \
             tc.tile_pool(name="rcp", bufs=8) as rcp:
            # qT/kT in per-512-col chunk tiles for fine dependency grain
            qT = [perm.tile([P, 512], BF, tag=f"qT{n}", name=f"qT{n}")
                  for n in range(T // 512)]
            kT = [perm.tile([P, 512], BF, tag=f"kT{n}", name=f"kT{n}")
                  for n in range(S // 512)]
            ones = perm.tile([P, 1], BF, tag="ones")
            nc.vector.memset(ones, 1.0)
            sums_d = dpool.tile([1, T], FP)

            # Coalesced input loads: the SP-queue DMA issue costs ~0.6us
            # per instruction, so small per-tile loads are issue-bound.
            # Each load below is one big DMA whose (k p) c -> p (k c)
            # rearrange stacks the 128-row DRAM tiles along the free dim.
            # Per-chunk destination tiles keep the dependency grain fine.
            memT = [perm.tile([P, KD * 512], BF, tag=f"m{n}", name=f"m{n}")
                    for n in range(S // 512)]
            memb = [perm.tile([P, 8 * D], BF, tag=f"n{h}", name=f"n{h}")
                    for h in range(2)]
            tgt = [perm.tile([P, KD * 512], BF, tag=f"t{n}", name=f"t{n}")
                   for n in range(T // 512)]
            wq = perm.tile([P, KD * R], BF, tag="wq")
            wk = perm.tile([P, KD * R], BF, tag="wk")
            w2 = [perm.tile([P, 4 * O], BF, tag=f"w2{h}", name=f"w2{h}")
                  for h in range(2)]

            # DMA order = consumption order
            def stk(dst, src_ap, blk):
                # one DMA: stack 128-row DRAM tiles along the SBUF free dim
                nc.sync.dma_start(
                    out=dst.rearrange("p (k c) -> p k c", c=blk),
                    in_=src_ap.rearrange("(k p) c -> p k c", p=P))

            stk(wq, wq_d, R)
            stk(wk, wk_d, R)
            stk(tgt[0], tgtT_d[:, 0:512], 512)
            stk(memT[0], memT_d[:, 0:512], 512)
            stk(tgt[1], tgtT_d[:, 512:1024], 512)
            for n in range(1, S // 512):
                stk(memT[n], memT_d[:, ts(n, 512)], 512)
            for h in range(2):
                stk(memb[h], mem_d[ts(h, 1024), :], D)
            for h in range(2):
                stk(w2[h], w2_d[ts(h, 512), :], O)

            def mem_sl(s, dt):      # lhsT [s-tile, d-tile] slice of mem
                return memb[s // 8][:, (s % 8) * D + dt * P:
                                    (s % 8) * D + (dt + 1) * P]

            def w2_sl(dt, oh):      # rhs [d-tile, o-chunk] slice of W2T
                return w2[dt // 4][:, (dt % 4) * O + oh * 512:
                                   (dt % 4) * O + (oh + 1) * 512]

            # ---- attention + output projection, per 512-col strip ----
            # Projections share the psc banks and are emitted interleaved
            # with the strip-0 score chunks they feed: the PE stream is
            # in-order, so ready score work must not sit behind kT chunks
            # that are still waiting on their memT DMA.
            with tc.tile_pool(name="psc", bufs=2, space="PSUM") as psc, \
                 tc.tile_pool(name="pszt", bufs=1, space="PSUM") as pszt, \
                 tc.tile_pool(name="pso", bufs=3, space="PSUM") as pso:
                zt_tags = ["zta", "ztb", "ztc"]
                ztc = 2  # round-robin counter over zt banks

                # PE warmup while the first DMAs land: HAM starts at
                # K=4/8 (1.2 GHz) and needs ~3.4us of activity to reach
                # full clock; these dummy MMs (deps: only the ones memset)
                # span the initial DMA wait so real MMs start warm.
                wup_sb = perm.tile([P, 128], BF, tag="wup_sb")
                nc.vector.memset(wup_sb, 1.0)
                wup = pszt.tile([P, TQ], FP, tag="ztc", name="wup")
                for _ in range(36):
                    nc.tensor.matmul(wup[0:1, 0:128], ones, wup_sb,
                                     start=True, stop=True)

                def proj(dst, w, src):
                    ps = psc.tile([P, 512], FP, tag="sc", name="proj_ps")
                    for k in range(KD):
                        nc.tensor.matmul(ps, w[:, ts(k, R)],
                                         src[:, ts(k, 512)],
                                         start=(k == 0), stop=(k == KD - 1))
                    nc.vector.tensor_copy(dst, ps)

                ex = [[None] * NS for _ in range(NQ)]
                sums_ps = [pszt.tile([P, TQ], FP, tag=zt_tags[q],
                                     name=f"sums{q}")
                           for q in range(NQ)]

                def head(q, s):
                    sc = psc.tile([P, TQ], FP, tag="sc", name="sc")
                    nc.tensor.matmul(sc, kT[s // 4][:, ts(s % 4, P)],
                                     qT[q], start=True, stop=True)
                    e_t = expp.tile([P, TQ], BF, tag=f"x{q}_{s}",
                                    name=f"x{q}_{s}")
                    nc.scalar.activation(e_t, sc, Exp, scale=float(SCALE))
                    ex[q][s] = e_t
                    nc.tensor.matmul(sums_ps[q][0:1, :], ones, e_t,
                                     start=(s == 0), stop=(s == NS - 1))

                proj(qT[0], wq, tgt[0])
                for ch in range(S // 512):
                    proj(kT[ch], wk, memT[ch])
                    for s in range(4 * ch, 4 * ch + 4):
                        head(0, s)
                    if ch == 0:
                        proj(qT[1], wq, tgt[1])
                for s in range(NS):
                    head(1, s)
                for q in range(NQ):
                    ssb = rcp.tile([1, TQ], FP, tag="ssb")
                    nc.vector.tensor_copy(ssb, sums_ps[q][0:1, :])
                    nc.sync.dma_start(out=sums_d[0:1, ts(q, TQ)], in_=ssb)

                # Phase 2 (per strip): ZT accumulation, then the output
                # projection; strip q's out MMs overlap strip q+1's Z.
                for q in range(NQ):
                    zt_sb = [ztsb.tile([P, TQ], BF, tag=f"zs{d}",
                                       name=f"zs{d}") for d in range(ND)]
                    for dt in range(ND):
                        zt_ps = pszt.tile([P, TQ], FP, tag=zt_tags[ztc % 3])
                        ztc += 1
                        for s in range(NS):
                            nc.tensor.matmul(zt_ps, mem_sl(s, dt),
                                             ex[q][s], start=(s == 0),
                                             stop=(s == NS - 1))
                        nc.vector.tensor_copy(zt_sb[dt], zt_ps)

                    # per-query reciprocal of the row sums, transposed into
                    # partition layout via a DRAM bounce
                    rcs = []
                    for tt in range(TQ // P):
                        tg = q * (TQ // P) + tt
                        sload = rcp.tile([P, 1], FP, tag="sl")
                        nc.sync.dma_start(
                            out=sload,
                            in_=sums_d[0:1, ts(tg, P)].rearrange("a b -> b a"))
                        rc = rcp.tile([P, 1], FP, tag="rc")
                        nc.vector.reciprocal(rc, sload)
                        rcs.append(rc)

                    # out[t,o] = sum_d ZT[d,t-tile].T @ W2T[d,o-chunk]
                    for tt in range(TQ // P):
                        tg = q * (TQ // P) + tt
                        for oh in range(O // 512):
                            po = pso.tile([P, 512], FP)
                            for dt in range(ND):
                                nc.tensor.matmul(po, zt_sb[dt][:, ts(tt, P)],
                                                 w2_sl(dt, oh),
                                                 start=(dt == 0),
                                                 stop=(dt == ND - 1))
                            ob = outp.tile([P, 512], BF)
                            # alternate the scale-copy between DVE and ACT
                            # (ACT is idle by the out phase) so the final
                            # drain chain isn't serialized on one engine
                            if (tt + oh) % 2 == 0:
                                nc.vector.tensor_scalar_mul(ob, po, rcs[tt])
                            else:
                                nc.scalar.activation(
                                    ob, po, mybir.ActivationFunctionType.Copy,
                                    scale=rcs[tt])
                            # alternate store queues so the kernel-tail
                            # DMA drain halves (exp stream is done by now)
                            eng = nc.sync if (tt + oh) % 2 == 0 else nc.scalar
                            eng.dma_start(out=out_d[ts(tg, P), ts(oh, 512)],
                                          in_=ob)
    return nc


def kernel(tgt, memory, Wq, Wk, Wv, Wo):
    """8-way data-parallel (batch x query-half) low-rank cross-attention
    on the 8 NeuronCores via the hand-written Bass/Tile program above."""
    global LAST_RESULT, _PROG

    tgt = np.asarray(tgt, dtype=np.float32)
    memory = np.asarray(memory, dtype=np.float32)
    bf = ml_dtypes.bfloat16

    WqT = np.ascontiguousarray(np.asarray(Wq, np.float32).T).astype(bf)
    WkT = np.ascontiguousarray(np.asarray(Wk, np.float32).T).astype(bf)
    # weight folding: W2 = Wo @ Wv, device consumes W2T = Wv.T @ Wo.T
    W2T = np.ascontiguousarray(
        np.asarray(Wv, np.float32).T @ np.asarray(Wo, np.float32).T
    ).astype(bf)

    # core c -> batch c//2, query-half c%2
    in_maps = []
    for c in range(8):
        b, h = divmod(c, 2)
        mem_b = memory[b]
        in_maps.append({
            "tgtT": np.ascontiguousarray(tgt[b, h * T:(h + 1) * T, :].T)
                      .astype(bf),                      # [D, T]
            "memT": np.ascontiguousarray(mem_b.T).astype(bf),  # [D, S]
            "mem": np.ascontiguousarray(mem_b).astype(bf),     # [S, D]
            "WqT": WqT, "WkT": WkT, "W2T": W2T,
        })

    if _PROG is None:
        _PROG = _build_program()
        _PROG.finalize()
    res = run_bass_kernel_spmd(_PROG, in_maps, core_ids=list(range(8)),
                               trace=TRACE)
    LAST_RESULT = res

    out = np.empty((B, T_FULL, O), dtype=np.float32)
    for c in range(8):
        b, h = divmod(c, 2)
        out[b, h * T:(h + 1) * T, :] = res.results[c]["out"].astype(np.float32)
    return out
